# revision 33
# baseline (speedup 1.0000x reference)
"""Trainium2 Bass kernel for nn_GaussianBasis (2D gaussian-splat sum rasterizer).

Math: out[c,d,h,w] = sum_n opacity_n * exp(-sigma_n(h,w)) * features[c,n,d]
where sigma is a per-gaussian quadratic form in pixel coords.

Strategy (v3):
  - Bin gaussians host-side into 8x16-px buckets with a sigma <= SIG_CUT
    cutoff ellipse (SIG_CUT=12 keeps every bucket <= 32 gaussians here;
    bigger buckets are split into extra slots over the same pixels - exact,
    since the rasterizer is linear and the host adds partials). Each of the
    8 cores owns one 32-row band = 64 buckets; four 32-gaussian slots pack
    one 128-partition tile -> T = 16 tiles/core, sigma/exp cols = pixels/4.
  - sigma over a bucket is a K=6 matmul vs phi = [x^2,y^2,xy,x,y,1] in
    bucket-centered quarter-integer coords (exact in fp16); W6 split hi/lo
    doubles K to 12 for fp32-grade accuracy at fp16 matmul speed. One
    matmul per tile: psum [128, 128px].
  - Feature einsum: per tile, two K=128 stacked matmuls at tile_position
    (0,0): lhsT [128, 96] holds slot 2p's features in rows 32(2p)..+32 ->
    cols 0:48 and slot 2p+1's in the next 32 rows -> cols 48:96 (all other
    rows zero), yielding psum [96, 128] per slot pair. (Off-diagonal PE
    tile_positions fault on hardware, so the zero-padded K=128 form is
    load-bearing, not a convenience.)
  - exp on ACT per exp-group [2,4,4,4,2] tiles (PSUM->SBUF fp16, one
    column per packed 4-gaussian-slot pixel); PSUM->SBUF fp16 convert-
    copies per slot-pair: DVE takes the first 5, ACT the last 3 after its
    exp chain drains (GPSIMD cannot read PSUM). Flush groups [4,4,6,2]
    tiles DMA out as fp16 from the SP/HWDGE queue as copies land; the
    small last group keeps the end-of-kernel chain short. Host scatters
    slot blocks into the image, adds split-bucket partials, and upcasts
    (tol 2e-2 >> fp16 error).
  - PE p-state warmup: dummy matmuls start the ramp clock early and bridge
    the w12 input-DMA latency.
"""

import sys

sys.path.insert(0, "/opt/trn_rl_repo")

import numpy as np
from contextlib import ExitStack

N, C, H, W = 2048, 16, 256, 256
NCORES = 8
BH = 32                        # band height per core
BKH, BKW = 8, 16               # bucket shape
BPX = BKH * BKW                # 128 px per bucket
NBR = BH // BKH                # 4 bucket rows per core
NBC = W // BKW                 # 16 bucket cols
SLOT = 32                      # gaussians per slot (quarter tile)
SIG_CUT = 12.0                 # exp(-12) ~ 6e-6: negligible vs output scale

# exp-group tile spans (ACT pipeline) and flush-group spans (scatter DMAs);
# built for T=16, recomputed in _build_program for other T
EXP_GROUPS = [2, 4, 4, 4, 2]
FLUSH_GROUPS = [4, 4, 6, 2]

_cached = {}


def _params(np_inputs):
    """Per-gaussian params (fp64 host): centers, quadratic coeffs, cutoff
    radii, opacity-folded features."""
    xyz_raw = np.asarray(np_inputs["xyz_raw"], dtype=np.float32)
    cholesky_raw = np.asarray(np_inputs["cholesky_raw"], dtype=np.float32)
    features = np.asarray(np_inputs["features"], dtype=np.float32)
    opacity = np.asarray(np_inputs["opacity"], dtype=np.float32)
    xy = np.tanh(xyz_raw.astype(np.float64))
    cx = 0.5 * (xy[:, 0] + 1.0) * W
    cy = 0.5 * (xy[:, 1] + 1.0) * H
    chol = cholesky_raw.astype(np.float64) + np.array([0.5, 0.0, 0.5])
    l1, l2, l3 = chol[:, 0], chol[:, 1], chol[:, 2]
    a = l1 * l1
    b = l1 * l2
    c = l2 * l2 + l3 * l3
    det = a * c - b * b
    Aq, Bq, Cq = 0.5 * (c / det), -b / det, 0.5 * (a / det)
    rx = np.sqrt(2.0 * SIG_CUT * a) + 1.0
    ry = np.sqrt(2.0 * SIG_CUT * c) + 1.0
    featw = features.astype(np.float64) * opacity[:, 0][None, :, None]
    featw = np.transpose(featw, (1, 0, 2)).reshape(N, C * 3)
    return cx, cy, Aq, Bq, Cq, rx, ry, featw


def _host_prep(cx, cy, Aq, Bq, Cq, rx, ry, featw):
    """Bin into 8x16 buckets, split >32 buckets into multiple slots, pack 4
    slots per tile. Returns (w12, feat, slotmap, T):
      w12 [core][12, BPX + T*128]  (phi in the first BPX cols)
      feat [core][128, T*192]      stacked slot-pair feature lhsT
      slotmap [core][T*4] -> bucket index (row*NBC+col) or -1
    """
    h_lo = np.floor(cy - ry).astype(int)
    h_hi = np.ceil(cy + ry).astype(int)
    w_lo = np.floor(cx - rx).astype(int)
    w_hi = np.ceil(cx + rx).astype(int)
    nrow = H // BKH
    buckets = [[[] for _ in range(NBC)] for _ in range(nrow)]
    for n in range(N):
        for bh in range(max(0, h_lo[n] // BKH), min(nrow, h_hi[n] // BKH + 1)):
            for bw in range(max(0, w_lo[n] // BKW), min(NBC, w_hi[n] // BKW + 1)):
                buckets[bh][bw].append(n)

    core_slots = []
    for core in range(NCORES):
        slots = []
        for r in range(NBR):
            for cidx in range(NBC):
                ns = buckets[core * NBR + r][cidx]
                for off in range(0, max(len(ns), 1), SLOT):
                    slots.append((r * NBC + cidx, ns[off:off + SLOT]))
        core_slots.append(slots)
    T = max((len(s) + 3) // 4 for s in core_slots)
    T += T % 2  # pair logic (feat/copy) needs an even tile count

    w12 = np.zeros((NCORES, 12, BPX + T * 128), dtype=np.float16)
    feat = np.zeros((NCORES, 128, T * 192), dtype=np.float16)
    slotmap = np.full((NCORES, T * 4), -1, dtype=np.int32)
    for core in range(NCORES):
        for si, (bid, ns) in enumerate(core_slots[core]):
            ns = np.array(ns, dtype=int)
            k = len(ns)
            slotmap[core, si] = bid
            if k == 0:
                continue
            t, sl = si // 4, si % 4
            r, cidx = bid // NBC, bid % NBC
            cxl = cx[ns] - cidx * BKW - BKW / 2
            cyl = cy[ns] - (core * NBR + r) * BKH - BKH / 2
            An, Bn, Cn = Aq[ns], Bq[ns], Cq[ns]
            W6 = np.stack(
                [
                    An,
                    Cn,
                    Bn,
                    -(2.0 * An * cxl + Bn * cyl),
                    -(2.0 * Cn * cyl + Bn * cxl),
                    An * cxl * cxl + Cn * cyl * cyl + Bn * cxl * cyl,
                ],
                0,
            )
            W_hi = W6.astype(np.float16)
            W_lo = (W6 - W_hi.astype(np.float64)).astype(np.float16)
            base = BPX + t * 128 + sl * SLOT
            w12[core, :6, base:base + k] = W_hi
            w12[core, 6:, base:base + k] = W_lo
            pair = sl // 2
            row0 = 64 * pair + 32 * (sl % 2)
            col0 = t * 192 + 96 * pair + 48 * (sl % 2)
            feat[core, row0:row0 + k, col0:col0 + 48] = featw[ns].astype(
                np.float16)

    xs = (np.arange(BKW) + 0.5 - BKW / 2).astype(np.float32)
    ys = (np.arange(BKH) + 0.5 - BKH / 2).astype(np.float32)
    Yg, Xg = np.meshgrid(ys, xs, indexing="ij")
    phi6 = np.stack(
        [Xg * Xg, Yg * Yg, Xg * Yg, Xg, Yg, np.ones_like(Xg)], 0
    ).reshape(6, BPX)
    w12[:, :, 0:BPX] = np.concatenate([phi6, phi6], 0).astype(np.float16)
    return w12, feat, slotmap, T


def _spans(sizes):
    out, lo = [], 0
    for s in sizes:
        out.append((lo, lo + s))
        lo += s
    return out


def _build_program(T):
    import concourse.bacc as bacc
    import concourse.tile as tile
    import concourse.mybir as mybir

    if T == 16:
        eg, fg = EXP_GROUPS, FLUSH_GROUPS
    else:
        eg = [2] + [4] * ((T - 4) // 4) + [2 + (T - 4) % 4]
        fg = [4] * (T // 4) + ([T % 4] if T % 4 else [])
    assert sum(eg) == T and sum(fg) == T
    egs, fgs = _spans(eg), _spans(fg)

    nc = bacc.Bacc("TRN2", target_bir_lowering=False, debug=False,
                   num_devices=NCORES)
    w12_ap = nc.dram_tensor("w12", [12, BPX + T * 128], mybir.dt.float16,
                            kind="ExternalInput").ap()
    feat_ap = nc.dram_tensor("feat", [128, T * 192], mybir.dt.float16,
                             kind="ExternalInput").ap()
    # out[p, t*256 + pair*128 + px]: p<48 -> channel p of slot 2*pair,
    # p>=48 -> channel p-48 of slot 2*pair+1. Host scatters + upcasts.
    out_ap = nc.dram_tensor("out", [96, T * 256], mybir.dt.float16,
                            kind="ExternalOutput").ap()

    with tile.TileContext(nc) as tc:
        with ExitStack() as ctx:
            consts = ctx.enter_context(tc.tile_pool(name="consts", bufs=1))
            spool = ctx.enter_context(
                tc.tile_pool(name="sig", bufs=2, space="PSUM"))
            opool = ctx.enter_context(
                tc.tile_pool(name="acc", bufs=6, space="PSUM"))
            gpool = ctx.enter_context(tc.tile_pool(name="g", bufs=4))

            # PE p-state warmup: the ramp clock starts at PE's first busy
            # moment; bridge until the w12 DMA lands (~3us)
            dummy = consts.tile([12, 384], mybir.dt.float16)
            nc.gpsimd.memset(dummy, 0)
            for _ in range(8):
                psum_w = spool.tile([128, 4 * BPX], mybir.dt.float32,
                                    name="ps")
                nc.tensor.matmul(psum_w[:, 0:256], dummy[:, 0:128],
                                 dummy[:, 128:384], start=True, stop=True)

            # fp16 staging tiles, one per flush group
            sts = []
            for i, (lo, hi) in enumerate(fgs):
                st = consts.tile([96, (hi - lo) * 256], mybir.dt.float16,
                                 name=f"st{i}")
                sts.append(st)

            # inputs: w12 + feat chunks on the SP HWDGE queue (Pool is busy
            # generating the output scatter descriptors)
            w12_sb = consts.tile([12, BPX + T * 128], mybir.dt.float16)
            nc.sync.dma_start(out=w12_sb, in_=w12_ap)
            phi_sb = w12_sb[:, 0:BPX]
            feat_sb = consts.tile([128, T * 192], mybir.dt.float16)
            FC1, FC2 = 2 * 192, 8 * 192
            nc.sync.dma_start(out=feat_sb[:, :FC1], in_=feat_ap[:, :FC1])
            nc.sync.dma_start(out=feat_sb[:, FC1:FC2], in_=feat_ap[:, FC1:FC2])
            nc.sync.dma_start(out=feat_sb[:, FC2:], in_=feat_ap[:, FC2:])


            psum_s = [None] * len(egs)
            g_sb = [None] * len(egs)
            psum_o = [None] * (T // 2 + 1)

            def sigma(gi):
                lo, hi = egs[gi]
                psum_s[gi] = spool.tile([128, 4 * BPX], mybir.dt.float32,
                                        name="ps")
                for j in range(hi - lo):
                    t = lo + j
                    nc.tensor.matmul(
                        psum_s[gi][:, j * BPX:(j + 1) * BPX],
                        w12_sb[:, BPX + t * 128:BPX + (t + 1) * 128],
                        phi_sb, start=True, stop=True)

            def expg(gi):
                lo, hi = egs[gi]
                n = hi - lo
                g_sb[gi] = gpool.tile([128, 4 * BPX], mybir.dt.float16,
                                      name="g")
                nc.scalar.activation(
                    g_sb[gi][:, 0:n * BPX], psum_s[gi][:, 0:n * BPX],
                    mybir.ActivationFunctionType.Exp, bias=0.0, scale=-1.0)

            def feat(t):
                pi, pj = t // 2, t % 2
                if pj == 0:
                    psum_o[pi] = opool.tile([96, 512], mybir.dt.float32,
                                            name="po")
                gi = next(i for i, (lo, hi) in enumerate(egs) if lo <= t < hi)
                gj = t - egs[gi][0]
                # K=128 with the complementary half of feat_sb zeroed: both
                # slot-pairs run at tile_position (0,0) (off-diagonal PE
                # tiles fault on hardware)
                for pair in range(2):
                    nc.tensor.matmul(
                        psum_o[pi][:, pj * 256 + pair * 128:
                                   pj * 256 + (pair + 1) * 128],
                        feat_sb[:, t * 192 + 96 * pair:
                                t * 192 + 96 * pair + 96],
                        g_sb[gi][:, gj * BPX:(gj + 1) * BPX],
                        start=True, stop=True)

            def copy_pair(pi, eng):
                t = 2 * pi
                fi = next(i for i, (lo, hi) in enumerate(fgs) if lo <= t < hi)
                lo = fgs[fi][0]
                dst = sts[fi][:, (t - lo) * 256:(t - lo + 2) * 256]
                src = psum_o[pi]
                if hasattr(eng, "tensor_copy"):
                    eng.tensor_copy(dst, src)
                else:
                    eng.copy(dst, src)

            # copy-engine plan: DVE takes even-ish tiles, Pool odd, ACT the
            # last tiles once its exp chain is done
            npair = T // 2
            pair_eng = {}
            for pi in range(npair):
                pair_eng[pi] = nc.vector if pi < npair - 3 else nc.scalar

            def flush(i):
                lo, hi = fgs[i]
                nc.sync.dma_start(
                    out=out_ap[:, lo * 256:hi * 256], in_=sts[i])

            fsched = {hi - 1: [i] for i, (lo, hi) in enumerate(fgs)}

            def maybe_flush(t):
                for i in fsched.get(t, []):
                    flush(i)

            # emission order = per-engine program order: all sigmas and
            # exps lead (ACT is the pipeline driver), feats/copies/flushes
            # trail in tile order
            for gi in range(len(egs)):
                sigma(gi)
                expg(gi)
            for t in range(T):
                feat(t)
                if t % 2 == 1:
                    copy_pair(t // 2, pair_eng[t // 2])
                maybe_flush(t)

    nc.compile()
    return nc


def kernel(xyz_raw, cholesky_raw, features, opacity):
    from concourse.bass_utils import run_bass_kernel_spmd

    np_inputs = {
        "xyz_raw": np.asarray(xyz_raw, dtype=np.float32),
        "cholesky_raw": np.asarray(cholesky_raw, dtype=np.float32),
        "features": np.asarray(features, dtype=np.float32),
        "opacity": np.asarray(opacity, dtype=np.float32),
    }
    w12, feat, slotmap, T = _host_prep(*_params(np_inputs))
    if T not in _cached:
        _cached[T] = _build_program(T)
    nc = _cached[T]
    in_maps = [{"w12": w12[b], "feat": feat[b]} for b in range(NCORES)]
    res = run_bass_kernel_spmd(nc, in_maps, core_ids=list(range(NCORES)))

    out = np.zeros((C * 3, H, W), dtype=np.float32)
    for core in range(NCORES):
        r = np.asarray(res.results[core]["out"], dtype=np.float32)
        for si in range(T * 4):
            bid = slotmap[core, si]
            if bid < 0:
                continue
            t, sl = si // 4, si % 4
            pair, half = sl // 2, sl % 2
            blk = r[48 * half:48 * half + 48,
                    t * 256 + pair * 128:t * 256 + (pair + 1) * 128]
            row = core * NBR + bid // NBC
            cw = bid % NBC
            out[:, row * BKH:(row + 1) * BKH, cw * BKW:(cw + 1) * BKW] += \
                blk.reshape(48, BKH, BKW)
    return out.reshape(C, 3, H, W)
